# revision 50
# baseline (speedup 1.0000x reference)
"""Multi-head causal attention with RoPE for TRN2, 8 NeuronCores.

Problem: B=2, T=2048, D=2048, 16 heads x head_dim 128, fp32.
  qkv = x @ Wqkv.T + bqkv ; RoPE(q, k) interleaved-pairs; causal softmax attention;
  out = attn_out @ Wo.T + bo.

Sharding: core c in 0..7 -> (batch b = c//4, head-group g = c%4 of 4 heads).
Each core computes its batch's partial output (its 4 heads' contribution through
the out-projection); host sums the 4 group partials per batch and adds bo.

v2 design (vs baseline): all matmul operands bf16 (fp32 PSUM accumulate);
q/k/v stay SBUF-resident (no DRAM roundtrip); attention inner loop is
head-round-robin with PV lagging one k-chunk (deep pipeline); softmax
denominator accumulated on DVE/Pool + gpsimd partition_all_reduce instead of
ones-matmuls on PE; out-projection of tile j-1 interleaved into tile j's
steps, draining PSUM directly to DRAM (attention) or via DVE copies (tail).
"""
import os
import sys

for _p in ("/opt/trn_rl_repo", "/root/.axon_site/_ro/trn_rl_repo"):
    if os.path.isdir(_p) and _p not in sys.path:
        sys.path.insert(0, _p)

import numpy as np

import concourse.bacc as bacc
import concourse.mybir as mybir
import concourse.tile as tile
from concourse import bass_isa
from concourse.bass_utils import run_bass_kernel_spmd

dt = mybir.dt
AF = mybir.ActivationFunctionType

B = 2
T = 2048
D = 2048
NH = 16
HD = 128
ROPE_BASE = 10000.0
N_CORES = 8
GROUPS = 4          # head-groups (tensor-parallel axis)
HPG = NH // GROUPS  # heads per group = 4
FQK = HPG * HD      # 512
FV = HPG * HD       # 512
QT = 512            # q-tile width in attention
NQT = T // QT       # 4
NKC = T // 128      # 16 k-chunks
NCC = D // 128      # 16 contraction chunks
TB = 1024           # phase-A t-block
NTB = T // TB       # 2
SCALE = 1.0 / float(np.sqrt(HD))


def _c0(j, kc):
    """First valid q column (within the tile) for k-chunk kc at q-tile j."""
    m = kc - 4 * j
    return 0 if m <= 0 else 128 * m


def build(loop=1):
    """Emit the per-core BIR program (identical for all 8 cores)."""
    import contextlib

    nc = bacc.Bacc("TRN2", target_bir_lowering=False, debug=False)

    xT_d = nc.dram_tensor("xT", [D, T], dt.bfloat16, kind="ExternalInput")
    wqp_d = nc.dram_tensor("wqpack", [8, 128, NCC * 128], dt.bfloat16,
                           kind="ExternalInput")
    wvp_d = nc.dram_tensor("wvpack", [128, NCC * FV], dt.bfloat16,
                           kind="ExternalInput")
    woT_d = nc.dram_tensor("woT", [FV, D], dt.bfloat16, kind="ExternalInput")
    cos_d = nc.dram_tensor("cosT", [HD, T], dt.bfloat16, kind="ExternalInput")
    sin_d = nc.dram_tensor("sinT", [HD, T], dt.bfloat16, kind="ExternalInput")
    mask_d = nc.dram_tensor("masks", [HD, 4, QT], dt.bfloat16, kind="ExternalInput")
    bqk_d = nc.dram_tensor("bqk", [2 * FQK, 1], dt.float32, kind="ExternalInput")
    bv_d = nc.dram_tensor("bvb", [HD, FV], dt.bfloat16, kind="ExternalInput")
    out_d = nc.dram_tensor("outp", [T, D], dt.bfloat16, kind="ExternalOutput")

    with tile.TileContext(nc, pool_alloc_mode="queue") as tc:
        with (
            tc.For_i(0, loop, 1) if loop > 1 else contextlib.nullcontext(),
            tc.tile_pool(name="res", bufs=1) as res,
        ):
            # persistent SBUF residents (bf16): k,q [dh, t] de-interleaved; v natural
            k_rs, q_rs = [], []
            for h in range(HPG):
                k_rs.append(res.tile([HD, T], dt.bfloat16, tag=f"kr{h}",
                                     name=f"kr_{h}"))
                q_rs.append(res.tile([HD, T], dt.bfloat16, tag=f"qr{h}",
                                     name=f"qr_{h}"))
            v_r = res.tile([128, NKC, HPG, HD], dt.bfloat16, tag="vr", name="v_r")

            # pools that must survive into phase B (tb1's v chains run there)
            es = contextlib.ExitStack()
            xpool = es.enter_context(tc.tile_pool(name="xb", bufs=1))
            wvpool = es.enter_context(tc.tile_pool(name="wvp", bufs=1))

            # -------- Phase A: qkv projection + RoPE on q,k --------
            with (
                tc.tile_pool(name="wq", bufs=1) as wpool,
                tc.tile_pool(name="arope", bufs=4) as arope,
                tc.tile_pool(name="abias", bufs=1) as abias,
                tc.tile_pool(name="aps", bufs=4, space="PSUM") as aps,
                tc.tile_pool(name="vps", bufs=2, space="PSUM") as vps,
            ):
                # SP (sync) queue, ordered so the first matmul chain starts
                # ASAP: first wq quarter, first x quarter, then interleaved.
                wq_blocks = [None] * 8
                wqr = wqp_d.ap().rearrange("fb p (cc f) -> fb p cc f", f=128)
                def load_wq(fb, csl=None):
                    if wq_blocks[fb] is None:
                        wq_blocks[fb] = wpool.tile(
                            [128, NCC, 128], dt.bfloat16, tag=f"wq{fb}",
                            name=f"wq_{fb}")
                    csl = csl if csl is not None else slice(0, NCC)
                    nc.sync.dma_start(
                        out=wq_blocks[fb][:, csl, :], in_=wqr[fb][:, csl, :],
                    )
                xr = xT_d.ap().rearrange("(cc p) t -> p cc t", p=128)
                xbs = []
                for tb in range(NTB):
                    xbs.append(xpool.tile([128, NCC, TB], dt.bfloat16,
                                          tag=f"xb{tb}", name=f"xb_{tb}"))
                def load_x0(q4):
                    if q4 == 0:
                        return  # loaded in two halves up front
                    nc.sync.dma_start(
                        out=xbs[0][:, q4 * 4:(q4 + 1) * 4, :],
                        in_=xr[:, q4 * 4:(q4 + 1) * 4, 0:TB],
                    )
                load_wq(0, slice(0, 4))
                nc.sync.dma_start(out=xbs[0][:, 0:2, :], in_=xr[:, 0:2, 0:TB])
                nc.sync.dma_start(out=xbs[0][:, 2:4, :], in_=xr[:, 2:4, 0:TB])
                load_wq(0, slice(4, NCC))
                load_x0(1)
                load_x0(2)
                load_x0(3)
                load_wq(1)
                bqk_sb = abias.tile([128, 8, 1], dt.float32)
                nc.sync.dma_start(
                    out=bqk_sb, in_=bqk_d.ap().rearrange("(f p) o -> p f o", p=128)
                )
                load_wq(2)
                cos_t = abias.tile([HD, T], dt.bfloat16)
                sin_t = abias.tile([HD, T], dt.bfloat16)
                nc.sync.dma_start(out=cos_t, in_=cos_d.ap())
                nc.sync.dma_start(out=sin_t, in_=sin_d.ap())
                load_wq(3)
                bv_sb = wvpool.tile([HD, FV], dt.bfloat16)
                nc.sync.dma_start(out=bv_sb, in_=bv_d.ap())
                for fb in range(4, 8):
                    load_wq(fb)
                wv_b = wvpool.tile([128, NCC, FV], dt.bfloat16)
                nc.sync.dma_start(
                    out=wv_b,
                    in_=wvp_d.ap().rearrange("p (cc f) -> p cc f", f=FV),
                )
                for tb in range(1, NTB):
                    nc.sync.dma_start(
                        out=xbs[tb], in_=xr[:, :, tb * TB:(tb + 1) * TB]
                    )

                def emit_qk(tb, f):
                    # qkv chains (two 512-wide half-chains: one matmul may
                    # only address a single PSUM bank) + RoPE drain.
                    # rotate-half is a partition-swap done by two SBUF-to-SBUF
                    # DMAs (sign folded in sin table).
                    xb = xbs[tb]
                    tsl = slice(tb * TB, (tb + 1) * TB)
                    s1 = arope.tile([128, TB], dt.bfloat16, tag="s1")
                    for hf in range(2):
                        csl = slice(hf * 512, (hf + 1) * 512)
                        ps = aps.tile([128, 512], dt.float32)
                        for cc in range(NCC):
                            nc.tensor.matmul(
                                ps,
                                wq_blocks[f][:, cc, :],
                                xb[:, cc, csl],
                                start=(cc == 0),
                                stop=(cc == NCC - 1),
                            )
                        nc.scalar.activation(out=s1[:, csl], in_=ps,
                                             func=AF.Identity,
                                             bias=bqk_sb[:, f, :])
                    half = HD // 2
                    rot = arope.tile([128, TB], dt.bfloat16, tag="rot")
                    nc.scalar.dma_start(out=rot[0:half, :], in_=s1[half:, :])
                    nc.scalar.dma_start(out=rot[half:, :], in_=s1[0:half, :])
                    nc.vector.tensor_mul(out=s1, in0=s1, in1=cos_t[:, tsl])
                    nc.vector.tensor_mul(out=rot, in0=rot, in1=sin_t[:, tsl])
                    dest = q_rs[f] if f < 4 else k_rs[f - 4]
                    nc.vector.tensor_add(out=dest[:, tsl], in0=s1, in1=rot)

                # tb0: qk blocks then its v chains (wv/x arrive over the
                # first ~25us); tb1: qk only — its v chains run inside
                # phase B as PE filler between attention steps.
                for f in range(8):
                    emit_qk(0, f)
                for ts in range(TB // 128):
                    psv = vps.tile([128, FV], dt.float32)
                    for cc in range(NCC):
                        nc.tensor.matmul(
                            psv,
                            xbs[0][:, cc, ts * 128:(ts + 1) * 128],
                            wv_b[:, cc, :],
                            start=(cc == 0),
                            stop=(cc == NCC - 1),
                        )
                    nc.vector.tensor_add(
                        out=v_r[:, ts, :, :], in0=psv, in1=bv_sb,
                    )
                for f in range(8):
                    emit_qk(1, f)

            # -------- Phase B + C: attention, out-proj per q-tile --------
            with (
                tc.tile_pool(name="bsing", bufs=1) as bsing,
                tc.tile_pool(name="pt", bufs=3) as ptp,
                tc.tile_pool(name="accp", bufs=1) as accp,
                tc.tile_pool(name="nrm", bufs=1) as nrmp,
                tc.tile_pool(name="osb", bufs=2) as osbp,
                tc.tile_pool(name="wo", bufs=1) as wop,
                tc.tile_pool(name="cdr", bufs=3) as cdrain,
                tc.tile_pool(name="ps_s", bufs=2, space="PSUM") as ps_s,
                tc.tile_pool(name="ps_o", bufs=1, space="PSUM") as ps_o,
                tc.tile_pool(name="cps", bufs=2, space="PSUM") as cps,
            ):
                mask_t = bsing.tile([HD, 4, QT], dt.bfloat16)
                nc.sync.dma_start(out=mask_t, in_=mask_d.ap())
                wo_sb = wop.tile([128, HPG, D], dt.bfloat16)
                nc.sync.dma_start(
                    out=wo_sb, in_=woT_d.ap().rearrange("(hh p) o -> p hh o", p=128)
                )

                crow = [None]  # current [128, D] bf16 drain row (per tt)

                def emit_cproj(pj, o_hs, groups):
                    # groups iterate (tt, oo) with oo innermost; the 4 oo
                    # drains share one [128, D] SBUF row flushed by one DMA.
                    for tt, oo in groups:
                        psc = cps.tile([128, QT], dt.float32,
                                       name=f"cps_{pj}_{tt}_{oo}", tag="cps")
                        for h in range(HPG):
                            nc.tensor.matmul(
                                psc,
                                o_hs[h][:, tt * 128:(tt + 1) * 128],
                                wo_sb[:, h, oo * QT:(oo + 1) * QT],
                                start=(h == 0), stop=(h == HPG - 1),
                            )
                        if oo == 0:
                            crow[0] = cdrain.tile([128, 4, QT], dt.bfloat16,
                                                  name=f"cdr_{pj}_{tt}", tag="cdr")
                        if oo % 2 == 0:
                            nc.vector.tensor_copy(out=crow[0][:, oo, :], in_=psc)
                        else:
                            nc.scalar.copy(out=crow[0][:, oo, :], in_=psc)
                        if oo % 2 == 1:  # flush half-rows to shorten the tail
                            r0 = pj * QT + tt * 128
                            csl = slice((oo - 1) * QT, (oo + 1) * QT)
                            nc.sync.dma_start(
                                out=out_d.ap()[r0:r0 + 128, csl],
                                in_=crow[0][:, oo - 1:oo + 1, :],
                            )

                def emit_v_b(kc):
                    # tb1 v chain, run as PE filler between attention steps
                    psv = cps.tile([128, FV], dt.float32, tag="cps",
                                   name=f"vb_{kc}")
                    ts = kc - TB // 128
                    for cc in range(NCC):
                        nc.tensor.matmul(
                            psv,
                            xbs[1][:, cc, ts * 128:(ts + 1) * 128],
                            wv_b[:, cc, :],
                            start=(cc == 0),
                            stop=(cc == NCC - 1),
                        )
                    nc.vector.tensor_add(
                        out=v_r[:, kc, :, :], in0=psv, in1=bv_sb,
                    )

                # tb1 v chains assigned as filler to the early steps of each
                # tile (kc list per tile index)
                vfill = {1: [8, 9, 10, 11], 2: [12, 13], 3: [14, 15], 0: []}

                # tile order: j=1 first (needs only tb0 data, no masks in its
                # first rows -> cheap pipeline fill at the A->B boundary);
                # all-diagonal j=0 last. prev tile's out-proj interleaves into
                # the current tile; a few groups are held back to cover each
                # tile's normalize latency.
                prev = None  # (pj, o_heads)
                for j in (1, 2, 3, 0):
                    nkc = 4 * (j + 1)
                    psum_o = [ps_o.tile([HD, QT], dt.float32, tag=f"po{h}",
                                        name=f"po_{j}_{h}") for h in range(HPG)]
                    accs = [accp.tile([128, QT], dt.float16, tag=f"acc{h}",
                                      name=f"acc_{j}_{h}") for h in range(HPG)]
                    o_heads = [osbp.tile([HD, QT], dt.bfloat16, tag=f"osb{h}",
                                         name=f"osb_{j}_{h}") for h in range(HPG)]
                    cgroups = ([(tt, oo) for tt in range(4) for oo in range(4)]
                               if prev is not None else [])
                    # hold back more groups when the current tile has few
                    # steps to spread them over (j=0: only 3 PV steps)
                    HOLD = 7 if j == 0 else 3
                    nspread = max(1, len(cgroups) - HOLD)
                    pts = {}

                    def s_row(kc):
                        c0 = _c0(j, kc)
                        for h in range(HPG):
                            psum_s = ps_s.tile([128, QT], dt.float32,
                                               name=f"s_{j}_{h}_{kc}", tag="ps_s")
                            nc.tensor.matmul(
                                psum_s[:, c0:],
                                k_rs[h][:, kc * 128:(kc + 1) * 128],
                                q_rs[h][:, j * QT + c0:(j + 1) * QT],
                                start=True, stop=True,
                            )
                            pt = ptp.tile([128, QT], dt.bfloat16, tag=f"pt{h}",
                                          name=f"pt_{j}_{h}_{kc}")
                            nc.scalar.activation(
                                out=pt[:, c0:], in_=psum_s[:, c0:],
                                func=AF.Exp, scale=SCALE,
                            )
                            m = kc - 4 * j
                            if m >= 0:
                                nc.vector.tensor_mul(
                                    out=pt[:, c0:], in0=pt[:, c0:],
                                    in1=mask_t[:, m, c0:],
                                )
                            eng = nc.vector if h < 2 else nc.gpsimd
                            if kc == 0:
                                eng.tensor_copy(out=accs[h], in_=pt)
                            else:
                                eng.tensor_add(out=accs[h][:, c0:],
                                               in0=accs[h][:, c0:],
                                               in1=pt[:, c0:])
                            pts[(h, kc)] = pt

                    def pv_row(kc):
                        c0 = _c0(j, kc)
                        for h in range(HPG):
                            nc.tensor.matmul(
                                psum_o[h][:, c0:],
                                v_r[:, kc, h, :],
                                pts.pop((h, kc))[:, c0:],
                                start=(kc == 0), stop=(kc == nkc - 1),
                            )

                    # lag-2 software pipeline: PV row kc-2 follows S row kc
                    def after_pv(r):
                        if not cgroups:
                            return
                        lo = min(nspread, nspread * r // (nkc - 1))
                        hi = min(nspread, nspread * (r + 1) // (nkc - 1))
                        for gi in range(lo, hi):
                            emit_cproj(prev[0], prev[1], [cgroups[gi]])

                    vq = list(vfill[j])
                    s_row(0)
                    s_row(1)
                    for kc in range(2, nkc):
                        s_row(kc)
                        pv_row(kc - 2)
                        if vq:
                            emit_v_b(vq.pop(0))
                        after_pv(kc - 2)
                    pv_row(nkc - 2)
                    after_pv(nkc - 2)
                    pv_row(nkc - 1)
                    if vq:
                        emit_v_b(vq.pop(0))
                    after_pv(nkc - 1)
                    # softmax denominators + normalize
                    for h in range(HPG):
                        ar = nrmp.tile([128, QT], dt.float32, tag=f"ar{h}",
                                       name=f"ar_{j}_{h}")
                        nc.gpsimd.partition_all_reduce(
                            ar, accs[h], channels=128,
                            reduce_op=bass_isa.ReduceOp.add,
                        )
                        nc.vector.reciprocal(out=ar, in_=ar)
                        nc.vector.tensor_mul(out=o_heads[h], in0=psum_o[h],
                                             in1=ar)
                    # held-back groups cover the normalize chain latency
                    for gi in range(nspread, len(cgroups)):
                        emit_cproj(prev[0], prev[1], [cgroups[gi]])
                    prev = (j, o_heads)
                # tail: last tile's out-projection
                emit_cproj(prev[0], prev[1],
                           [(tt, oo) for tt in range(4) for oo in range(4)])
            es.close()
    nc.compile()
    return nc


# ---------------------------------------------------------------------------
# Host side
# ---------------------------------------------------------------------------

_DEINT = np.concatenate([np.arange(0, HD, 2), np.arange(1, HD, 2)])  # de-interleave


def _rope_tables():
    half = HD // 2
    inv_freq = 1.0 / (ROPE_BASE ** (np.arange(half, dtype=np.float64) / half))
    t = np.arange(T, dtype=np.float64)
    fr = t[None, :] * inv_freq[:, None]          # (64, T)
    cos = np.concatenate([np.cos(fr), np.cos(fr)], axis=0)
    sin = np.concatenate([-np.sin(fr), np.sin(fr)], axis=0)
    return cos, sin


def _masks():
    # [k_local, m, q_local]: 1 where k_local + 128*m <= q_local
    m = np.zeros((HD, 4, QT), dtype=np.float64)
    kk = np.arange(HD)[:, None]
    qq = np.arange(QT)[None, :]
    for i in range(4):
        m[:, i, :] = (kk <= qq - 128 * i)
    return m


def _bf16(a):
    import ml_dtypes
    return np.asarray(a).astype(ml_dtypes.bfloat16)


def make_in_maps(x, Wqkv, bqkv, Wo, bo):
    cos, sin = _rope_tables()
    masks = _masks()

    Wq = Wqkv[0 * D:1 * D]
    Wk = Wqkv[1 * D:2 * D]
    Wv = Wqkv[2 * D:3 * D]
    bq = bqkv[0 * D:1 * D]
    bk = bqkv[1 * D:2 * D]
    bv = bqkv[2 * D:3 * D]

    in_maps = []
    for c in range(N_CORES):
        b, g = divmod(c, GROUPS)
        hsl = slice(g * HPG * HD, (g + 1) * HPG * HD)
        # de-interleaved row order for q,k heads of this group
        rows = np.arange(g * HPG * HD, (g + 1) * HPG * HD).reshape(HPG, HD)
        rows = rows[:, _DEINT].reshape(-1)

        wq = Wq[rows]                       # (512, D)
        wk = Wk[rows]
        wv = Wv[hsl]                        # natural order
        wqkT = np.concatenate([wq, wk], axis=0).T.astype(np.float64)  # (D, 1024)
        wqpack = np.ascontiguousarray(
            wqkT.reshape(NCC, 128, 8, 128)      # (cc, p, fb, f)
                .transpose(2, 1, 0, 3)           # (fb, p, cc, f)
                .reshape(8, 128, NCC * 128)
        )
        wvT = wv.T.astype(np.float64)            # (D, 512)
        wvpack = np.ascontiguousarray(
            wvT.reshape(NCC, 128, FV).transpose(1, 0, 2).reshape(128, NCC * FV)
        )
        woT = np.ascontiguousarray(Wo[:, hsl].T.astype(np.float64))  # (512, D)

        bqk = np.concatenate([bq[rows], bk[rows]]).astype(np.float32)[:, None]
        bvb = np.broadcast_to(np.asarray(bv[hsl]), (HD, FV)).copy()

        xT = np.ascontiguousarray(np.asarray(x[b]).T)  # (D, T)

        in_maps.append({
            "xT": _bf16(xT),
            "wqpack": _bf16(wqpack),
            "wvpack": _bf16(wvpack),
            "woT": _bf16(woT),
            "cosT": _bf16(cos),
            "sinT": _bf16(sin),
            "masks": _bf16(masks),
            "bqk": bqk,
            "bvb": _bf16(bvb),
        })
    return in_maps


_NC_CACHE = {}


def _get_nc(loop=1):
    if loop not in _NC_CACHE:
        _NC_CACHE[loop] = build(loop=loop)
    return _NC_CACHE[loop]


def kernel(x, Wqkv, bqkv, Wo, bo):
    x = np.asarray(x)
    Wqkv = np.asarray(Wqkv)
    bqkv = np.asarray(bqkv)
    Wo = np.asarray(Wo)
    bo = np.asarray(bo)

    nc = _get_nc()
    in_maps = make_in_maps(x, Wqkv, bqkv, Wo, bo)
    res = run_bass_kernel_spmd(nc, in_maps, core_ids=list(range(N_CORES)))

    out = np.zeros((B, T, D), dtype=np.float32)
    for c in range(N_CORES):
        b = c // GROUPS
        out[b] += np.asarray(res.results[c]["outp"], dtype=np.float32)
    out += bo.astype(np.float32)[None, None, :]
    return out


# revision 52
# speedup vs baseline: 1.4450x; 1.4450x over previous
"""Multi-head causal attention with RoPE for TRN2, 8 NeuronCores.

Problem: B=2, T=2048, D=2048, 16 heads x head_dim 128, fp32.
  qkv = x @ Wqkv.T + bqkv ; RoPE(q, k) interleaved-pairs; causal softmax attention;
  out = attn_out @ Wo.T + bo.

Sharding: core c in 0..7 -> (batch b = c//4, head-group g = c%4 of 4 heads).
Each core computes its batch's partial output (its 4 heads' contribution through
the out-projection); host sums the 4 group partials per batch and adds bo.

v2 design (vs baseline): all matmul operands bf16 (fp32 PSUM accumulate);
q/k/v stay SBUF-resident (no DRAM roundtrip); attention inner loop is
head-round-robin with PV lagging one k-chunk (deep pipeline); softmax
denominator accumulated on DVE/Pool + gpsimd partition_all_reduce instead of
ones-matmuls on PE; out-projection of tile j-1 interleaved into tile j's
steps, draining PSUM directly to DRAM (attention) or via DVE copies (tail).
"""
import os
import sys

for _p in ("/opt/trn_rl_repo", "/root/.axon_site/_ro/trn_rl_repo"):
    if os.path.isdir(_p) and _p not in sys.path:
        sys.path.insert(0, _p)

import numpy as np

import concourse.bacc as bacc
import concourse.mybir as mybir
import concourse.tile as tile
from concourse import bass_isa
from concourse.bass_utils import run_bass_kernel_spmd

dt = mybir.dt
AF = mybir.ActivationFunctionType

B = 2
T = 2048
D = 2048
NH = 16
HD = 128
ROPE_BASE = 10000.0
N_CORES = 8
GROUPS = 4          # head-groups (tensor-parallel axis)
HPG = NH // GROUPS  # heads per group = 4
FQK = HPG * HD      # 512
FV = HPG * HD       # 512
QT = 512            # q-tile width in attention
NQT = T // QT       # 4
NKC = T // 128      # 16 k-chunks
NCC = D // 128      # 16 contraction chunks
TB = 1024           # phase-A t-block
NTB = T // TB       # 2
SCALE = 1.0 / float(np.sqrt(HD))


def _c0(j, kc):
    """First valid q column (within the tile) for k-chunk kc at q-tile j."""
    m = kc - 4 * j
    return 0 if m <= 0 else 128 * m


def build(loop=1):
    """Emit the per-core BIR program (identical for all 8 cores)."""
    import contextlib

    nc = bacc.Bacc("TRN2", target_bir_lowering=False, debug=False)

    xT_d = nc.dram_tensor("xT", [D, T], dt.bfloat16, kind="ExternalInput")
    wqp_d = nc.dram_tensor("wqpack", [8, 128, NCC * 128], dt.bfloat16,
                           kind="ExternalInput")
    wvp_d = nc.dram_tensor("wvpack", [128, NCC * FV], dt.bfloat16,
                           kind="ExternalInput")
    woT_d = nc.dram_tensor("woT", [FV, D], dt.bfloat16, kind="ExternalInput")
    cos_d = nc.dram_tensor("cosT", [HD, T], dt.bfloat16, kind="ExternalInput")
    sin_d = nc.dram_tensor("sinT", [HD, T], dt.bfloat16, kind="ExternalInput")
    mask_d = nc.dram_tensor("masks", [HD, 4, QT], dt.bfloat16, kind="ExternalInput")
    bqk_d = nc.dram_tensor("bqk", [2 * FQK, 1], dt.float32, kind="ExternalInput")
    bv_d = nc.dram_tensor("bvb", [HD, FV], dt.bfloat16, kind="ExternalInput")
    out_d = nc.dram_tensor("outp", [T, D], dt.bfloat16, kind="ExternalOutput")

    with tile.TileContext(nc, pool_alloc_mode="queue") as tc:
        with (
            tc.For_i(0, loop, 1) if loop > 1 else contextlib.nullcontext(),
            tc.tile_pool(name="res", bufs=1) as res,
        ):
            # persistent SBUF residents (bf16): k,q [dh, t] de-interleaved; v natural
            k_rs, q_rs = [], []
            for h in range(HPG):
                k_rs.append(res.tile([HD, T], dt.bfloat16, tag=f"kr{h}",
                                     name=f"kr_{h}"))
                q_rs.append(res.tile([HD, T], dt.bfloat16, tag=f"qr{h}",
                                     name=f"qr_{h}"))
            v_r = res.tile([128, NKC, HPG, HD], dt.bfloat16, tag="vr", name="v_r")

            # pools that must survive into phase B (tb1's v chains run there)
            es = contextlib.ExitStack()
            xpool = es.enter_context(tc.tile_pool(name="xb", bufs=1))
            wvpool = es.enter_context(tc.tile_pool(name="wvp", bufs=1))

            # -------- Phase A: qkv projection + RoPE on q,k --------
            with (
                tc.tile_pool(name="wq", bufs=1) as wpool,
                tc.tile_pool(name="arope", bufs=4) as arope,
                tc.tile_pool(name="abias", bufs=1) as abias,
                tc.tile_pool(name="aps", bufs=4, space="PSUM") as aps,
                tc.tile_pool(name="vps", bufs=2, space="PSUM") as vps,
            ):
                # SP (sync) queue, ordered so the first matmul chain starts
                # ASAP: first wq quarter, first x quarter, then interleaved.
                wq_blocks = [None] * 8
                wqr = wqp_d.ap().rearrange("fb p (cc f) -> fb p cc f", f=128)
                def load_wq(fb, csl=None):
                    if wq_blocks[fb] is None:
                        wq_blocks[fb] = wpool.tile(
                            [128, NCC, 128], dt.bfloat16, tag=f"wq{fb}",
                            name=f"wq_{fb}")
                    csl = csl if csl is not None else slice(0, NCC)
                    nc.sync.dma_start(
                        out=wq_blocks[fb][:, csl, :], in_=wqr[fb][:, csl, :],
                    )
                xr = xT_d.ap().rearrange("(cc p) t -> p cc t", p=128)
                xbs = []
                for tb in range(NTB):
                    xbs.append(xpool.tile([128, NCC, TB], dt.bfloat16,
                                          tag=f"xb{tb}", name=f"xb_{tb}"))
                def load_x0(q4):
                    nc.sync.dma_start(
                        out=xbs[0][:, q4 * 4:(q4 + 1) * 4, :],
                        in_=xr[:, q4 * 4:(q4 + 1) * 4, 0:TB],
                    )
                load_wq(0, slice(0, 4))
                load_x0(0)
                load_wq(0, slice(4, NCC))
                load_x0(1)
                load_x0(2)
                load_x0(3)
                load_wq(1)
                bqk_sb = abias.tile([128, 8, 1], dt.float32)
                nc.sync.dma_start(
                    out=bqk_sb, in_=bqk_d.ap().rearrange("(f p) o -> p f o", p=128)
                )
                load_wq(2)
                cos_t = abias.tile([HD, T], dt.bfloat16)
                sin_t = abias.tile([HD, T], dt.bfloat16)
                nc.sync.dma_start(out=cos_t, in_=cos_d.ap())
                nc.sync.dma_start(out=sin_t, in_=sin_d.ap())
                load_wq(3)
                bv_sb = wvpool.tile([HD, FV], dt.bfloat16)
                nc.sync.dma_start(out=bv_sb, in_=bv_d.ap())
                for fb in range(4, 8):
                    load_wq(fb)
                wv_b = wvpool.tile([128, NCC, FV], dt.bfloat16)
                nc.sync.dma_start(
                    out=wv_b,
                    in_=wvp_d.ap().rearrange("p (cc f) -> p cc f", f=FV),
                )
                for tb in range(1, NTB):
                    nc.sync.dma_start(
                        out=xbs[tb], in_=xr[:, :, tb * TB:(tb + 1) * TB]
                    )

                def emit_qk(tb, f):
                    # qkv chains (two 512-wide half-chains: one matmul may
                    # only address a single PSUM bank) + RoPE drain.
                    # rotate-half is a partition-swap done by two SBUF-to-SBUF
                    # DMAs (sign folded in sin table).
                    xb = xbs[tb]
                    tsl = slice(tb * TB, (tb + 1) * TB)
                    s1 = arope.tile([128, TB], dt.bfloat16, tag="s1")
                    for hf in range(2):
                        csl = slice(hf * 512, (hf + 1) * 512)
                        ps = aps.tile([128, 512], dt.float32)
                        for cc in range(NCC):
                            nc.tensor.matmul(
                                ps,
                                wq_blocks[f][:, cc, :],
                                xb[:, cc, csl],
                                start=(cc == 0),
                                stop=(cc == NCC - 1),
                            )
                        nc.scalar.activation(out=s1[:, csl], in_=ps,
                                             func=AF.Identity,
                                             bias=bqk_sb[:, f, :])
                    half = HD // 2
                    rot = arope.tile([128, TB], dt.bfloat16, tag="rot")
                    nc.scalar.dma_start(out=rot[0:half, :], in_=s1[half:, :])
                    nc.scalar.dma_start(out=rot[half:, :], in_=s1[0:half, :])
                    nc.vector.tensor_mul(out=s1, in0=s1, in1=cos_t[:, tsl])
                    nc.vector.tensor_mul(out=rot, in0=rot, in1=sin_t[:, tsl])
                    dest = q_rs[f] if f < 4 else k_rs[f - 4]
                    nc.vector.tensor_add(out=dest[:, tsl], in0=s1, in1=rot)

                # tb0: qk blocks then its v chains (wv/x arrive over the
                # first ~25us); tb1: qk only — its v chains run inside
                # phase B as PE filler between attention steps.
                for f in range(8):
                    emit_qk(0, f)
                for ts in range(TB // 128):
                    psv = vps.tile([128, FV], dt.float32)
                    for cc in range(NCC):
                        nc.tensor.matmul(
                            psv,
                            xbs[0][:, cc, ts * 128:(ts + 1) * 128],
                            wv_b[:, cc, :],
                            start=(cc == 0),
                            stop=(cc == NCC - 1),
                        )
                    nc.vector.tensor_add(
                        out=v_r[:, ts, :, :], in0=psv, in1=bv_sb,
                    )
                for f in range(8):
                    emit_qk(1, f)

            # -------- Phase B + C: attention, out-proj per q-tile --------
            with (
                tc.tile_pool(name="bsing", bufs=1) as bsing,
                tc.tile_pool(name="pt", bufs=3) as ptp,
                tc.tile_pool(name="accp", bufs=1) as accp,
                tc.tile_pool(name="nrm", bufs=1) as nrmp,
                tc.tile_pool(name="osb", bufs=2) as osbp,
                tc.tile_pool(name="wo", bufs=1) as wop,
                tc.tile_pool(name="cdr", bufs=3) as cdrain,
                tc.tile_pool(name="ps_s", bufs=2, space="PSUM") as ps_s,
                tc.tile_pool(name="ps_o", bufs=1, space="PSUM") as ps_o,
                tc.tile_pool(name="cps", bufs=2, space="PSUM") as cps,
            ):
                mask_t = bsing.tile([HD, 4, QT], dt.bfloat16)
                nc.sync.dma_start(out=mask_t, in_=mask_d.ap())
                wo_sb = wop.tile([128, HPG, D], dt.bfloat16)
                nc.sync.dma_start(
                    out=wo_sb, in_=woT_d.ap().rearrange("(hh p) o -> p hh o", p=128)
                )

                crow = [None]  # current [128, D] bf16 drain row (per tt)

                def emit_cproj(pj, o_hs, groups):
                    # groups iterate (tt, oo) with oo innermost; the 4 oo
                    # drains share one [128, D] SBUF row flushed by one DMA.
                    for tt, oo in groups:
                        psc = cps.tile([128, QT], dt.float32,
                                       name=f"cps_{pj}_{tt}_{oo}", tag="cps")
                        for h in range(HPG):
                            nc.tensor.matmul(
                                psc,
                                o_hs[h][:, tt * 128:(tt + 1) * 128],
                                wo_sb[:, h, oo * QT:(oo + 1) * QT],
                                start=(h == 0), stop=(h == HPG - 1),
                            )
                        if oo == 0:
                            crow[0] = cdrain.tile([128, 4, QT], dt.bfloat16,
                                                  name=f"cdr_{pj}_{tt}", tag="cdr")
                        if oo % 2 == 0:
                            nc.vector.tensor_copy(out=crow[0][:, oo, :], in_=psc)
                        else:
                            nc.scalar.copy(out=crow[0][:, oo, :], in_=psc)
                        if oo == 3:
                            r0 = pj * QT + tt * 128
                            nc.sync.dma_start(
                                out=out_d.ap()[r0:r0 + 128, :], in_=crow[0],
                            )

                def emit_v_b(kc):
                    # tb1 v chain, run as PE filler between attention steps
                    psv = cps.tile([128, FV], dt.float32, tag="cps",
                                   name=f"vb_{kc}")
                    ts = kc - TB // 128
                    for cc in range(NCC):
                        nc.tensor.matmul(
                            psv,
                            xbs[1][:, cc, ts * 128:(ts + 1) * 128],
                            wv_b[:, cc, :],
                            start=(cc == 0),
                            stop=(cc == NCC - 1),
                        )
                    nc.vector.tensor_add(
                        out=v_r[:, kc, :, :], in0=psv, in1=bv_sb,
                    )

                # tb1 v chains assigned as filler to the early steps of each
                # tile (kc list per tile index)
                vfill = {1: [8, 9, 10, 11], 2: [12, 13], 3: [14, 15], 0: []}

                # tile order: j=1 first (needs only tb0 data, no masks in its
                # first rows -> cheap pipeline fill at the A->B boundary);
                # all-diagonal j=0 last. prev tile's out-proj interleaves into
                # the current tile; a few groups are held back to cover each
                # tile's normalize latency.
                HOLD = 3
                prev = None  # (pj, o_heads)
                for j in (1, 2, 3, 0):
                    nkc = 4 * (j + 1)
                    psum_o = [ps_o.tile([HD, QT], dt.float32, tag=f"po{h}",
                                        name=f"po_{j}_{h}") for h in range(HPG)]
                    accs = [accp.tile([128, QT], dt.float16, tag=f"acc{h}",
                                      name=f"acc_{j}_{h}") for h in range(HPG)]
                    o_heads = [osbp.tile([HD, QT], dt.bfloat16, tag=f"osb{h}",
                                         name=f"osb_{j}_{h}") for h in range(HPG)]
                    cgroups = ([(tt, oo) for tt in range(4) for oo in range(4)]
                               if prev is not None else [])
                    nspread = max(1, len(cgroups) - HOLD)
                    pts = {}

                    def s_row(kc):
                        c0 = _c0(j, kc)
                        for h in range(HPG):
                            psum_s = ps_s.tile([128, QT], dt.float32,
                                               name=f"s_{j}_{h}_{kc}", tag="ps_s")
                            nc.tensor.matmul(
                                psum_s[:, c0:],
                                k_rs[h][:, kc * 128:(kc + 1) * 128],
                                q_rs[h][:, j * QT + c0:(j + 1) * QT],
                                start=True, stop=True,
                            )
                            pt = ptp.tile([128, QT], dt.bfloat16, tag=f"pt{h}",
                                          name=f"pt_{j}_{h}_{kc}")
                            nc.scalar.activation(
                                out=pt[:, c0:], in_=psum_s[:, c0:],
                                func=AF.Exp, scale=SCALE,
                            )
                            m = kc - 4 * j
                            if m >= 0:
                                nc.vector.tensor_mul(
                                    out=pt[:, c0:], in0=pt[:, c0:],
                                    in1=mask_t[:, m, c0:],
                                )
                            eng = nc.vector
                            if kc == 0:
                                eng.tensor_copy(out=accs[h], in_=pt)
                            else:
                                eng.tensor_add(out=accs[h][:, c0:],
                                               in0=accs[h][:, c0:],
                                               in1=pt[:, c0:])
                            pts[(h, kc)] = pt

                    def pv_row(kc):
                        c0 = _c0(j, kc)
                        for h in range(HPG):
                            nc.tensor.matmul(
                                psum_o[h][:, c0:],
                                v_r[:, kc, h, :],
                                pts.pop((h, kc))[:, c0:],
                                start=(kc == 0), stop=(kc == nkc - 1),
                            )

                    # lag-2 software pipeline: PV row kc-2 follows S row kc
                    def after_pv(r):
                        if not cgroups:
                            return
                        lo = min(nspread, nspread * r // (nkc - 1))
                        hi = min(nspread, nspread * (r + 1) // (nkc - 1))
                        for gi in range(lo, hi):
                            emit_cproj(prev[0], prev[1], [cgroups[gi]])

                    vq = list(vfill[j])
                    s_row(0)
                    s_row(1)
                    for kc in range(2, nkc):
                        s_row(kc)
                        pv_row(kc - 2)
                        if vq:
                            emit_v_b(vq.pop(0))
                        after_pv(kc - 2)
                    pv_row(nkc - 2)
                    after_pv(nkc - 2)
                    pv_row(nkc - 1)
                    if vq:
                        emit_v_b(vq.pop(0))
                    after_pv(nkc - 1)
                    # softmax denominators + normalize
                    for h in range(HPG):
                        ar = nrmp.tile([128, QT], dt.float32, tag=f"ar{h}",
                                       name=f"ar_{j}_{h}")
                        nc.gpsimd.partition_all_reduce(
                            ar, accs[h], channels=128,
                            reduce_op=bass_isa.ReduceOp.add,
                        )
                        nc.vector.reciprocal(out=ar, in_=ar)
                        nc.vector.tensor_mul(out=o_heads[h], in0=psum_o[h],
                                             in1=ar)
                    # held-back groups cover the normalize chain latency
                    for gi in range(nspread, len(cgroups)):
                        emit_cproj(prev[0], prev[1], [cgroups[gi]])
                    prev = (j, o_heads)
                # tail: last tile's out-projection
                emit_cproj(prev[0], prev[1],
                           [(tt, oo) for tt in range(4) for oo in range(4)])
            es.close()
    nc.compile()
    return nc


# ---------------------------------------------------------------------------
# Host side
# ---------------------------------------------------------------------------

_DEINT = np.concatenate([np.arange(0, HD, 2), np.arange(1, HD, 2)])  # de-interleave


def _rope_tables():
    half = HD // 2
    inv_freq = 1.0 / (ROPE_BASE ** (np.arange(half, dtype=np.float64) / half))
    t = np.arange(T, dtype=np.float64)
    fr = t[None, :] * inv_freq[:, None]          # (64, T)
    cos = np.concatenate([np.cos(fr), np.cos(fr)], axis=0)
    sin = np.concatenate([-np.sin(fr), np.sin(fr)], axis=0)
    return cos, sin


def _masks():
    # [k_local, m, q_local]: 1 where k_local + 128*m <= q_local
    m = np.zeros((HD, 4, QT), dtype=np.float64)
    kk = np.arange(HD)[:, None]
    qq = np.arange(QT)[None, :]
    for i in range(4):
        m[:, i, :] = (kk <= qq - 128 * i)
    return m


def _bf16(a):
    import ml_dtypes
    return np.asarray(a).astype(ml_dtypes.bfloat16)


def make_in_maps(x, Wqkv, bqkv, Wo, bo):
    cos, sin = _rope_tables()
    masks = _masks()

    Wq = Wqkv[0 * D:1 * D]
    Wk = Wqkv[1 * D:2 * D]
    Wv = Wqkv[2 * D:3 * D]
    bq = bqkv[0 * D:1 * D]
    bk = bqkv[1 * D:2 * D]
    bv = bqkv[2 * D:3 * D]

    in_maps = []
    for c in range(N_CORES):
        b, g = divmod(c, GROUPS)
        hsl = slice(g * HPG * HD, (g + 1) * HPG * HD)
        # de-interleaved row order for q,k heads of this group
        rows = np.arange(g * HPG * HD, (g + 1) * HPG * HD).reshape(HPG, HD)
        rows = rows[:, _DEINT].reshape(-1)

        wq = Wq[rows]                       # (512, D)
        wk = Wk[rows]
        wv = Wv[hsl]                        # natural order
        wqkT = np.concatenate([wq, wk], axis=0).T.astype(np.float64)  # (D, 1024)
        wqpack = np.ascontiguousarray(
            wqkT.reshape(NCC, 128, 8, 128)      # (cc, p, fb, f)
                .transpose(2, 1, 0, 3)           # (fb, p, cc, f)
                .reshape(8, 128, NCC * 128)
        )
        wvT = wv.T.astype(np.float64)            # (D, 512)
        wvpack = np.ascontiguousarray(
            wvT.reshape(NCC, 128, FV).transpose(1, 0, 2).reshape(128, NCC * FV)
        )
        woT = np.ascontiguousarray(Wo[:, hsl].T.astype(np.float64))  # (512, D)

        bqk = np.concatenate([bq[rows], bk[rows]]).astype(np.float32)[:, None]
        bvb = np.broadcast_to(np.asarray(bv[hsl]), (HD, FV)).copy()

        xT = np.ascontiguousarray(np.asarray(x[b]).T)  # (D, T)

        in_maps.append({
            "xT": _bf16(xT),
            "wqpack": _bf16(wqpack),
            "wvpack": _bf16(wvpack),
            "woT": _bf16(woT),
            "cosT": _bf16(cos),
            "sinT": _bf16(sin),
            "masks": _bf16(masks),
            "bqk": bqk,
            "bvb": _bf16(bvb),
        })
    return in_maps


_NC_CACHE = {}


def _get_nc(loop=1):
    if loop not in _NC_CACHE:
        _NC_CACHE[loop] = build(loop=loop)
    return _NC_CACHE[loop]


def kernel(x, Wqkv, bqkv, Wo, bo):
    x = np.asarray(x)
    Wqkv = np.asarray(Wqkv)
    bqkv = np.asarray(bqkv)
    Wo = np.asarray(Wo)
    bo = np.asarray(bo)

    nc = _get_nc()
    in_maps = make_in_maps(x, Wqkv, bqkv, Wo, bo)
    res = run_bass_kernel_spmd(nc, in_maps, core_ids=list(range(N_CORES)))

    out = np.zeros((B, T, D), dtype=np.float32)
    for c in range(N_CORES):
        b = c // GROUPS
        out[b] += np.asarray(res.results[c]["outp"], dtype=np.float32)
    out += bo.astype(np.float32)[None, None, :]
    return out


# revision 56
# speedup vs baseline: 1.5017x; 1.0392x over previous
"""Multi-head causal attention with RoPE for TRN2, 8 NeuronCores.

Problem: B=2, T=2048, D=2048, 16 heads x head_dim 128, fp32.
  qkv = x @ Wqkv.T + bqkv ; RoPE(q, k) interleaved-pairs; causal softmax attention;
  out = attn_out @ Wo.T + bo.

Sharding: core c in 0..7 -> (batch b = c//4, head-group g = c%4 of 4 heads).
Each core computes its batch's partial output (its 4 heads' contribution through
the out-projection); host sums the 4 group partials per batch and adds bo.

v2 design (vs baseline): all matmul operands bf16 (fp32 PSUM accumulate),
bf16 output partials upcast on host; q/k/v stay SBUF-resident (no DRAM
roundtrip); phase-A t-blocks of 1024 with half-chain matmuls (PSUM bank
limit); rotate-half via two SBUF-to-SBUF DMAs (sign folded into the sin
table); attention is head-round-robin with PV lagging two k-chunks; softmax
denominators accumulate on DVE (fp16) + one gpsimd partition_all_reduce per
(tile, head) instead of ones-matmuls on PE; tb1's v-projection chains run
inside phase B as PE filler; out-projection of the previous tile interleaves
into the current tile's steps (tile order 1,2,3,0), drained via DVE/ACT
copies into row buffers flushed by one DMA per 128-row stripe.
"""
import os
import sys

for _p in ("/opt/trn_rl_repo", "/root/.axon_site/_ro/trn_rl_repo"):
    if os.path.isdir(_p) and _p not in sys.path:
        sys.path.insert(0, _p)

import numpy as np

import concourse.bacc as bacc
import concourse.mybir as mybir
import concourse.tile as tile
from concourse import bass_isa
from concourse.bass_utils import run_bass_kernel_spmd

dt = mybir.dt
AF = mybir.ActivationFunctionType

B = 2
T = 2048
D = 2048
NH = 16
HD = 128
ROPE_BASE = 10000.0
N_CORES = 8
GROUPS = 4          # head-groups (tensor-parallel axis)
HPG = NH // GROUPS  # heads per group = 4
FQK = HPG * HD      # 512
FV = HPG * HD       # 512
QT = 512            # q-tile width in attention
NQT = T // QT       # 4
NKC = T // 128      # 16 k-chunks
NCC = D // 128      # 16 contraction chunks
TB = 1024           # phase-A t-block
NTB = T // TB       # 2
SCALE = 1.0 / float(np.sqrt(HD))


def _c0(j, kc):
    """First valid q column (within the tile) for k-chunk kc at q-tile j."""
    m = kc - 4 * j
    return 0 if m <= 0 else 128 * m


def build(loop=1):
    """Emit the per-core BIR program (identical for all 8 cores)."""
    import contextlib

    nc = bacc.Bacc("TRN2", target_bir_lowering=False, debug=False)

    xT_d = nc.dram_tensor("xT", [D, T], dt.bfloat16, kind="ExternalInput")
    wqp_d = nc.dram_tensor("wqpack", [8, 128, NCC * 128], dt.bfloat16,
                           kind="ExternalInput")
    wvp_d = nc.dram_tensor("wvpack", [128, NCC * FV], dt.bfloat16,
                           kind="ExternalInput")
    woT_d = nc.dram_tensor("woT", [FV, D], dt.bfloat16, kind="ExternalInput")
    cos_d = nc.dram_tensor("cosT", [HD, T], dt.bfloat16, kind="ExternalInput")
    sin_d = nc.dram_tensor("sinT", [HD, T], dt.bfloat16, kind="ExternalInput")
    mask_d = nc.dram_tensor("masks", [HD, 4, QT], dt.bfloat16, kind="ExternalInput")
    bqk_d = nc.dram_tensor("bqk", [2 * FQK, 1], dt.float32, kind="ExternalInput")
    bv_d = nc.dram_tensor("bvb", [HD, FV], dt.bfloat16, kind="ExternalInput")
    ones_d = nc.dram_tensor("ones", [HD, 1], dt.bfloat16, kind="ExternalInput")
    out_d = nc.dram_tensor("outp", [T, D], dt.bfloat16, kind="ExternalOutput")

    with tile.TileContext(nc, pool_alloc_mode="queue") as tc:
        with (
            tc.For_i(0, loop, 1) if loop > 1 else contextlib.nullcontext(),
            tc.tile_pool(name="res", bufs=1) as res,
        ):
            # persistent SBUF residents (bf16): k,q [dh, t] de-interleaved; v natural
            k_rs, q_rs = [], []
            for h in range(HPG):
                k_rs.append(res.tile([HD, T], dt.bfloat16, tag=f"kr{h}",
                                     name=f"kr_{h}"))
                q_rs.append(res.tile([HD, T], dt.bfloat16, tag=f"qr{h}",
                                     name=f"qr_{h}"))
            v_r = res.tile([128, NKC, HPG, HD], dt.bfloat16, tag="vr", name="v_r")

            # pools that must survive into phase B (tb1's v chains run there)
            es = contextlib.ExitStack()
            xpool = es.enter_context(tc.tile_pool(name="xb", bufs=1))
            wvpool = es.enter_context(tc.tile_pool(name="wvp", bufs=1))

            # -------- Phase A: qkv projection + RoPE on q,k --------
            with (
                tc.tile_pool(name="wq", bufs=1) as wpool,
                tc.tile_pool(name="arope", bufs=4) as arope,
                tc.tile_pool(name="abias", bufs=1) as abias,
                tc.tile_pool(name="aps", bufs=4, space="PSUM") as aps,
                tc.tile_pool(name="vps", bufs=2, space="PSUM") as vps,
            ):
                # SP (sync) queue, ordered so the first matmul chain starts
                # ASAP: first wq quarter, first x quarter, then interleaved.
                wq_blocks = [None] * 8
                wqr = wqp_d.ap().rearrange("fb p (cc f) -> fb p cc f", f=128)
                def load_wq(fb, csl=None):
                    if wq_blocks[fb] is None:
                        wq_blocks[fb] = wpool.tile(
                            [128, NCC, 128], dt.bfloat16, tag=f"wq{fb}",
                            name=f"wq_{fb}")
                    csl = csl if csl is not None else slice(0, NCC)
                    nc.sync.dma_start(
                        out=wq_blocks[fb][:, csl, :], in_=wqr[fb][:, csl, :],
                    )
                xr = xT_d.ap().rearrange("(cc p) t -> p cc t", p=128)
                xbs = []
                for tb in range(NTB):
                    xbs.append(xpool.tile([128, NCC, TB], dt.bfloat16,
                                          tag=f"xb{tb}", name=f"xb_{tb}"))
                def load_x0(q4):
                    nc.sync.dma_start(
                        out=xbs[0][:, q4 * 4:(q4 + 1) * 4, :],
                        in_=xr[:, q4 * 4:(q4 + 1) * 4, 0:TB],
                    )
                load_wq(0, slice(0, 4))
                load_x0(0)
                load_wq(0, slice(4, NCC))
                load_x0(1)
                load_x0(2)
                load_x0(3)
                load_wq(1)
                bqk_sb = abias.tile([128, 8, 1], dt.float32)
                nc.sync.dma_start(
                    out=bqk_sb, in_=bqk_d.ap().rearrange("(f p) o -> p f o", p=128)
                )
                load_wq(2)
                cos_t = abias.tile([HD, T], dt.bfloat16)
                sin_t = abias.tile([HD, T], dt.bfloat16)
                nc.sync.dma_start(out=cos_t, in_=cos_d.ap())
                nc.sync.dma_start(out=sin_t, in_=sin_d.ap())
                load_wq(3)
                bv_sb = wvpool.tile([HD, FV], dt.bfloat16)
                nc.sync.dma_start(out=bv_sb, in_=bv_d.ap())
                for fb in range(4, 8):
                    load_wq(fb)
                wv_b = wvpool.tile([128, NCC, FV], dt.bfloat16)
                nc.sync.dma_start(
                    out=wv_b,
                    in_=wvp_d.ap().rearrange("p (cc f) -> p cc f", f=FV),
                )
                for tb in range(1, NTB):
                    nc.sync.dma_start(
                        out=xbs[tb], in_=xr[:, :, tb * TB:(tb + 1) * TB]
                    )

                def emit_qk(tb, f):
                    # qkv chains (two 512-wide half-chains: one matmul may
                    # only address a single PSUM bank) + RoPE drain.
                    # rotate-half is a partition-swap done by two SBUF-to-SBUF
                    # DMAs (sign folded in sin table).
                    xb = xbs[tb]
                    tsl = slice(tb * TB, (tb + 1) * TB)
                    s1 = arope.tile([128, TB], dt.bfloat16, tag="s1")
                    for hf in range(2):
                        csl = slice(hf * 512, (hf + 1) * 512)
                        ps = aps.tile([128, 512], dt.float32)
                        for cc in range(NCC):
                            nc.tensor.matmul(
                                ps,
                                wq_blocks[f][:, cc, :],
                                xb[:, cc, csl],
                                start=(cc == 0),
                                stop=(cc == NCC - 1),
                            )
                        nc.scalar.activation(out=s1[:, csl], in_=ps,
                                             func=AF.Identity,
                                             bias=bqk_sb[:, f, :])
                    half = HD // 2
                    rot = arope.tile([128, TB], dt.bfloat16, tag="rot")
                    nc.scalar.dma_start(out=rot[0:half, :], in_=s1[half:, :])
                    nc.scalar.dma_start(out=rot[half:, :], in_=s1[0:half, :])
                    nc.vector.tensor_mul(out=s1, in0=s1, in1=cos_t[:, tsl])
                    nc.vector.tensor_mul(out=rot, in0=rot, in1=sin_t[:, tsl])
                    dest = q_rs[f] if f < 4 else k_rs[f - 4]
                    nc.vector.tensor_add(out=dest[:, tsl], in0=s1, in1=rot)

                # tb0: qk blocks then its v chains (wv/x arrive over the
                # first ~25us); tb1: qk only — its v chains run inside
                # phase B as PE filler between attention steps.
                for f in range(8):
                    emit_qk(0, f)
                for ts in range(TB // 128):
                    psv = vps.tile([128, FV], dt.float32)
                    for cc in range(NCC):
                        nc.tensor.matmul(
                            psv,
                            xbs[0][:, cc, ts * 128:(ts + 1) * 128],
                            wv_b[:, cc, :],
                            start=(cc == 0),
                            stop=(cc == NCC - 1),
                        )
                    nc.vector.tensor_add(
                        out=v_r[:, ts, :, :], in0=psv, in1=bv_sb,
                    )
                for f in range(8):
                    emit_qk(1, f)

            # -------- Phase B + C: attention, out-proj per q-tile --------
            with (
                tc.tile_pool(name="bsing", bufs=1) as bsing,
                tc.tile_pool(name="pt", bufs=3) as ptp,
                tc.tile_pool(name="accp", bufs=1) as accp,
                tc.tile_pool(name="nrm", bufs=1) as nrmp,
                tc.tile_pool(name="osb", bufs=2) as osbp,
                tc.tile_pool(name="wo", bufs=1) as wop,
                tc.tile_pool(name="cdr", bufs=3) as cdrain,
                tc.tile_pool(name="ps_s", bufs=2, space="PSUM") as ps_s,
                tc.tile_pool(name="ps_o", bufs=1, space="PSUM") as ps_o,
                tc.tile_pool(name="cps", bufs=2, space="PSUM") as cps,
            ):
                mask_t = bsing.tile([HD, 4, QT], dt.bfloat16)
                nc.sync.dma_start(out=mask_t, in_=mask_d.ap())
                ones_t = bsing.tile([HD, 1], dt.bfloat16)
                nc.sync.dma_start(out=ones_t, in_=ones_d.ap())
                wo_sb = wop.tile([128, HPG, D], dt.bfloat16)
                nc.sync.dma_start(
                    out=wo_sb, in_=woT_d.ap().rearrange("(hh p) o -> p hh o", p=128)
                )

                crow = [None]  # current [128, D] bf16 drain row (per tt)

                def emit_cproj(pj, o_hs, groups):
                    # groups iterate (tt, oo) with oo innermost; the 4 oo
                    # drains share one [128, D] SBUF row flushed by one DMA.
                    for tt, oo in groups:
                        psc = cps.tile([128, QT], dt.float32,
                                       name=f"cps_{pj}_{tt}_{oo}", tag="cps")
                        for h in range(HPG):
                            nc.tensor.matmul(
                                psc,
                                o_hs[h][:, tt * 128:(tt + 1) * 128],
                                wo_sb[:, h, oo * QT:(oo + 1) * QT],
                                start=(h == 0), stop=(h == HPG - 1),
                            )
                        if oo == 0:
                            crow[0] = cdrain.tile([128, 4, QT], dt.bfloat16,
                                                  name=f"cdr_{pj}_{tt}", tag="cdr")
                        if oo % 2 == 0:
                            nc.vector.tensor_copy(out=crow[0][:, oo, :], in_=psc)
                        else:
                            nc.scalar.copy(out=crow[0][:, oo, :], in_=psc)
                        if oo == 3:
                            r0 = pj * QT + tt * 128
                            nc.sync.dma_start(
                                out=out_d.ap()[r0:r0 + 128, :], in_=crow[0],
                            )

                def emit_v_b(kc):
                    # tb1 v chain, run as PE filler between attention steps
                    psv = cps.tile([128, FV], dt.float32, tag="cps",
                                   name=f"vb_{kc}")
                    ts = kc - TB // 128
                    for cc in range(NCC):
                        nc.tensor.matmul(
                            psv,
                            xbs[1][:, cc, ts * 128:(ts + 1) * 128],
                            wv_b[:, cc, :],
                            start=(cc == 0),
                            stop=(cc == NCC - 1),
                        )
                    nc.vector.tensor_add(
                        out=v_r[:, kc, :, :], in0=psv, in1=bv_sb,
                    )

                # tb1 v chains assigned as filler to the early steps of each
                # tile (kc list per tile index)
                vfill = {1: [8, 9, 10, 11], 2: [12, 13], 3: [14, 15], 0: []}

                # tile order: j=1 first (needs only tb0 data, no masks in its
                # first rows -> cheap pipeline fill at the A->B boundary);
                # all-diagonal j=0 last. prev tile's out-proj interleaves into
                # the current tile; a few groups are held back to cover each
                # tile's normalize latency.
                HOLD = 3
                prev = None  # (pj, o_heads)
                for j in (1, 2, 3, 0):
                    nkc = 4 * (j + 1)
                    psum_o = [ps_o.tile([HD, QT], dt.float32, tag=f"po{h}",
                                        name=f"po_{j}_{h}") for h in range(HPG)]
                    accs = [accp.tile([128, QT], dt.float16, tag=f"acc{h}",
                                      name=f"acc_{j}_{h}") for h in range(HPG)]
                    o_heads = [osbp.tile([HD, QT], dt.bfloat16, tag=f"osb{h}",
                                         name=f"osb_{j}_{h}") for h in range(HPG)]
                    cgroups = ([(tt, oo) for tt in range(4) for oo in range(4)]
                               if prev is not None else [])
                    nspread = max(1, len(cgroups) - HOLD)
                    pts = {}

                    def s_row(kc):
                        c0 = _c0(j, kc)
                        for h in range(HPG):
                            psum_s = ps_s.tile([128, QT], dt.float32,
                                               name=f"s_{j}_{h}_{kc}", tag="ps_s")
                            nc.tensor.matmul(
                                psum_s[:, c0:],
                                k_rs[h][:, kc * 128:(kc + 1) * 128],
                                q_rs[h][:, j * QT + c0:(j + 1) * QT],
                                start=True, stop=True,
                            )
                            pt = ptp.tile([128, QT], dt.bfloat16, tag=f"pt{h}",
                                          name=f"pt_{j}_{h}_{kc}")
                            nc.scalar.activation(
                                out=pt[:, c0:], in_=psum_s[:, c0:],
                                func=AF.Exp, scale=SCALE,
                            )
                            m = kc - 4 * j
                            if m >= 0:
                                nc.vector.tensor_mul(
                                    out=pt[:, c0:], in0=pt[:, c0:],
                                    in1=mask_t[:, m, c0:],
                                )
                            eng = nc.vector
                            if kc == 0:
                                eng.tensor_copy(out=accs[h], in_=pt)
                            else:
                                eng.tensor_add(out=accs[h][:, c0:],
                                               in0=accs[h][:, c0:],
                                               in1=pt[:, c0:])
                            pts[(h, kc)] = pt

                    def pv_row(kc):
                        c0 = _c0(j, kc)
                        for h in range(HPG):
                            nc.tensor.matmul(
                                psum_o[h][:, c0:],
                                v_r[:, kc, h, :],
                                pts.pop((h, kc))[:, c0:],
                                start=(kc == 0), stop=(kc == nkc - 1),
                            )

                    # lag-2 software pipeline: PV row kc-2 follows S row kc
                    def after_pv(r):
                        if not cgroups:
                            return
                        lo = min(nspread, nspread * r // (nkc - 1))
                        hi = min(nspread, nspread * (r + 1) // (nkc - 1))
                        for gi in range(lo, hi):
                            emit_cproj(prev[0], prev[1], [cgroups[gi]])

                    vq = list(vfill[j])
                    s_row(0)
                    s_row(1)
                    for kc in range(2, nkc):
                        s_row(kc)
                        pv_row(kc - 2)
                        if vq:
                            emit_v_b(vq.pop(0))
                        after_pv(kc - 2)
                    pv_row(nkc - 2)
                    after_pv(nkc - 2)
                    pv_row(nkc - 1)
                    if vq:
                        emit_v_b(vq.pop(0))
                    after_pv(nkc - 1)
                    # softmax denominators + normalize: one-shot ones-matmul
                    # per head reduces its fp16 accumulator across partitions
                    # into psum region [0:1], reused serially (the region
                    # read by reciprocal gates the next head's write)
                    psum_l = cps.tile([128, QT], dt.float32, tag="cps",
                                      name=f"pl_{j}")
                    held = list(range(nspread, len(cgroups)))
                    for h in range(HPG):
                        nc.tensor.matmul(psum_l[0:1, :], ones_t, accs[h],
                                         start=True, stop=True)
                        # held-back out-proj groups fill PE while the
                        # region-serialized reciprocal chain completes
                        if held:
                            emit_cproj(prev[0], prev[1], [cgroups[held.pop(0)]])
                        rc = nrmp.tile([1, QT], dt.float32, tag=f"rc{h}",
                                       name=f"rc_{j}_{h}")
                        nc.vector.reciprocal(out=rc, in_=psum_l[0:1, :])
                        bc = nrmp.tile([128, QT], dt.float32, tag=f"bc{h}",
                                       name=f"bc_{j}_{h}")
                        nc.gpsimd.partition_broadcast(bc, rc)
                        nc.vector.tensor_mul(out=o_heads[h], in0=psum_o[h],
                                             in1=bc)
                    for gi in held:
                        emit_cproj(prev[0], prev[1], [cgroups[gi]])
                    prev = (j, o_heads)
                # tail: last tile's out-projection
                emit_cproj(prev[0], prev[1],
                           [(tt, oo) for tt in range(4) for oo in range(4)])
            es.close()
    nc.compile()
    return nc


# ---------------------------------------------------------------------------
# Host side
# ---------------------------------------------------------------------------

_DEINT = np.concatenate([np.arange(0, HD, 2), np.arange(1, HD, 2)])  # de-interleave


def _rope_tables():
    half = HD // 2
    inv_freq = 1.0 / (ROPE_BASE ** (np.arange(half, dtype=np.float64) / half))
    t = np.arange(T, dtype=np.float64)
    fr = t[None, :] * inv_freq[:, None]          # (64, T)
    cos = np.concatenate([np.cos(fr), np.cos(fr)], axis=0)
    sin = np.concatenate([-np.sin(fr), np.sin(fr)], axis=0)
    return cos, sin


def _masks():
    # [k_local, m, q_local]: 1 where k_local + 128*m <= q_local
    m = np.zeros((HD, 4, QT), dtype=np.float64)
    kk = np.arange(HD)[:, None]
    qq = np.arange(QT)[None, :]
    for i in range(4):
        m[:, i, :] = (kk <= qq - 128 * i)
    return m


def _bf16(a):
    import ml_dtypes
    return np.asarray(a).astype(ml_dtypes.bfloat16)


def make_in_maps(x, Wqkv, bqkv, Wo, bo):
    cos, sin = _rope_tables()
    masks = _masks()

    Wq = Wqkv[0 * D:1 * D]
    Wk = Wqkv[1 * D:2 * D]
    Wv = Wqkv[2 * D:3 * D]
    bq = bqkv[0 * D:1 * D]
    bk = bqkv[1 * D:2 * D]
    bv = bqkv[2 * D:3 * D]

    in_maps = []
    for c in range(N_CORES):
        b, g = divmod(c, GROUPS)
        hsl = slice(g * HPG * HD, (g + 1) * HPG * HD)
        # de-interleaved row order for q,k heads of this group
        rows = np.arange(g * HPG * HD, (g + 1) * HPG * HD).reshape(HPG, HD)
        rows = rows[:, _DEINT].reshape(-1)

        wq = Wq[rows]                       # (512, D)
        wk = Wk[rows]
        wv = Wv[hsl]                        # natural order
        wqkT = np.concatenate([wq, wk], axis=0).T.astype(np.float64)  # (D, 1024)
        wqpack = np.ascontiguousarray(
            wqkT.reshape(NCC, 128, 8, 128)      # (cc, p, fb, f)
                .transpose(2, 1, 0, 3)           # (fb, p, cc, f)
                .reshape(8, 128, NCC * 128)
        )
        wvT = wv.T.astype(np.float64)            # (D, 512)
        wvpack = np.ascontiguousarray(
            wvT.reshape(NCC, 128, FV).transpose(1, 0, 2).reshape(128, NCC * FV)
        )
        woT = np.ascontiguousarray(Wo[:, hsl].T.astype(np.float64))  # (512, D)

        bqk = np.concatenate([bq[rows], bk[rows]]).astype(np.float32)[:, None]
        bvb = np.broadcast_to(np.asarray(bv[hsl]), (HD, FV)).copy()

        xT = np.ascontiguousarray(np.asarray(x[b]).T)  # (D, T)

        in_maps.append({
            "xT": _bf16(xT),
            "wqpack": _bf16(wqpack),
            "wvpack": _bf16(wvpack),
            "woT": _bf16(woT),
            "cosT": _bf16(cos),
            "sinT": _bf16(sin),
            "masks": _bf16(masks),
            "bqk": bqk,
            "bvb": _bf16(bvb),
            "ones": _bf16(np.ones((HD, 1))),
        })
    return in_maps


_NC_CACHE = {}


def _get_nc(loop=1):
    if loop not in _NC_CACHE:
        _NC_CACHE[loop] = build(loop=loop)
    return _NC_CACHE[loop]


def kernel(x, Wqkv, bqkv, Wo, bo):
    x = np.asarray(x)
    Wqkv = np.asarray(Wqkv)
    bqkv = np.asarray(bqkv)
    Wo = np.asarray(Wo)
    bo = np.asarray(bo)

    nc = _get_nc()
    in_maps = make_in_maps(x, Wqkv, bqkv, Wo, bo)
    res = run_bass_kernel_spmd(nc, in_maps, core_ids=list(range(N_CORES)))

    out = np.zeros((B, T, D), dtype=np.float32)
    for c in range(N_CORES):
        b = c // GROUPS
        out[b] += np.asarray(res.results[c]["outp"], dtype=np.float32)
    out += bo.astype(np.float32)[None, None, :]
    return out
